# revision 10
# baseline (speedup 1.0000x reference)
"""Trainium2 Bass kernel for GCNBlock (spectral-norm linear + GCN aggregation +
InstanceNorm + LeakyReLU) distributed across 8 NeuronCores.

v2 strategy (evolved from the dma_gather baseline after trace analysis):
  - out = (A @ x) @ WnT per dst tile; dst nodes sharded 8 ways (49 tiles of
    128 per core).
  - Gather of x rows per edge stays on gpsimd dma_gather (the only
    descriptor-rate-viable indexed path), edges partitioned by dst and
    sorted by src, split by int16 index halves, chunked 7 tiles per gather.
  - Self loops are NOT gathered: each tile's own x rows are a contiguous
    static HWDGE dma_start; their diag(coef) scatter block is part of S.
  - The one-hot scatter matrices S (including coef) are built ON HOST and
    streamed in via sync-engine DMA — the DVE tensor_scalar build used by the
    baseline costs ~2.4us per block (per-partition scalar-pointer reads) and
    was a co-bottleneck with the gather.
  - InstanceNorm epilogue avoids bn_stats/bn_aggr (1.8/4.5us per call) and
    per-partition-pointer DVE ops: ACT Square+accum gives sum(x^2), DVE
    tensor_reduce gives sum(x), tiny [128,1] ops produce rstd and
    -mean*rstd, and one ACT activation(Lrelu, scale=rstd, bias=-mean*rstd,
    alpha=0.2) applies normalize+LeakyReLU fused.
"""

import numpy as np
import ml_dtypes
from contextlib import ExitStack

import concourse.tile as tile
from concourse import bacc, mybir
from concourse.bass_utils import run_bass_kernel_spmd

# Problem constants (hardcoded per spec)
N, E, C = 50000, 800000, 128
P = 128
NCORES = 8
TPC = 49                # dst tiles per core
NPC = TPC * P           # 6272 dst nodes per core
NPAD = NCORES * NPC     # 50176 padded node count
HALF = 32768            # int16 index split point
CHUNK_TILES = 7
NCHUNKS = -(-TPC // CHUNK_TILES)  # 7
EPS_IN = 1e-5


def _preprocess(x, edge_index, W, b, u):
    """Host-side prep: spectral norm, edge partitioning, S-matrix build."""
    x = np.asarray(x, dtype=np.float32)
    ei = np.asarray(edge_index)
    W = np.asarray(W, dtype=np.float32)
    b = np.asarray(b, dtype=np.float32)
    u = np.asarray(u, dtype=np.float32)

    # --- spectral norm (one power iteration), matches reference ---
    eps = np.float32(1e-12)
    v = (W.T @ u).astype(np.float32)
    v = v / (np.float32(np.linalg.norm(v)) + eps)
    Wv = (W @ v).astype(np.float32)
    u2 = Wv / (np.float32(np.linalg.norm(Wv)) + eps)
    sigma = np.float32(u2 @ Wv)
    WnT = np.ascontiguousarray((W / sigma).T, dtype=np.float32)  # [cin, cout]

    src = ei[0].astype(np.int64)
    dst = ei[1].astype(np.int64)

    # --- degrees / coefficients (with self loops) ---
    deg = (np.bincount(dst, minlength=N) + 1).astype(np.float32)
    dinv = (1.0 / np.sqrt(deg)).astype(np.float32)
    dinv_pad = np.ones(NPAD, dtype=np.float32)
    dinv_pad[:N] = dinv
    coef = dinv[src] * dinv[dst]

    # --- group regular edges by (core, tile, src-half), sorted by src ---
    core = dst // NPC
    tile_g = (dst % NPC) // P
    dstloc = (dst % P).astype(np.int64)
    half = (src >= HALF).astype(np.int64)
    key = ((core * TPC + tile_g) * 2 + half).astype(np.int64)
    NG = NCORES * TPC * 2
    # sort by (key, src) so each group's gather addresses ascend
    order = np.lexsort((src, key))
    counts = np.bincount(key, minlength=NG)
    starts = np.zeros(NG + 1, dtype=np.int64)
    np.cumsum(counts, out=starts[1:])
    rank = np.arange(len(key), dtype=np.int64) - starts[key[order]]

    cnt3 = counts.reshape(NCORES, TPC, 2)
    nb = np.ceil(cnt3.max(axis=0) / P).astype(np.int64)  # [TPC, 2] gather blocks

    # block layout: chunk-major, then half, then tile within chunk
    blk_off = np.zeros((TPC, 2), dtype=np.int64)
    gather_blk0 = np.zeros((NCHUNKS, 2), dtype=np.int64)
    gather_nblk = np.zeros((NCHUNKS, 2), dtype=np.int64)
    pos = 0
    for c in range(NCHUNKS):
        t0 = c * CHUNK_TILES
        t1 = min(t0 + CHUNK_TILES, TPC)
        for h in range(2):
            gather_blk0[c, h] = pos
            for t in range(t0, t1):
                blk_off[t, h] = pos
                pos += nb[t, h]
            gather_nblk[c, h] = pos - gather_blk0[c, h]
    totblk = pos

    # gather indices (int16, wrapped) + S matrices for gather blocks
    IDXALL = np.zeros((NCORES, totblk * P), dtype=np.int16)
    # S layout: per tile t: [1 self block | nb[t,0] low blocks | nb[t,1] high]
    nbt = nb.sum(axis=1) + 1          # total S blocks per tile
    s_off = np.zeros(TPC + 1, dtype=np.int64)
    np.cumsum(nbt, out=s_off[1:])
    tot_s = int(s_off[-1])
    S = np.zeros((NCORES, P, tot_s * P), dtype=np.float32)

    o_core = core[order]
    o_tile = tile_g[order]
    o_half = half[order]
    o_blk = blk_off[o_tile, o_half] + rank // P
    o_slot = rank % P
    # S column index for edge: tile base + (1 + local block) * P + dstloc
    loc_blk = o_blk - blk_off[o_tile, 0]          # local gather-block id within tile
    # for high half, local id continues after low blocks:
    loc_blk = np.where(o_half == 1, nb[o_tile, 0] + (o_blk - blk_off[o_tile, 1]), loc_blk)
    s_col = (s_off[o_tile] + 1 + loc_blk) * P + dstloc[order]

    IDXALL[o_core, o_blk * P + o_slot] = (src[order] - o_half * HALF).astype(np.int16)
    S[o_core, o_slot, s_col] = coef[order]

    # self-loop diag blocks
    for t in range(TPC):
        cols = (s_off[t] * P) + np.arange(P)
        for ci in range(NCORES):
            nodes = ci * NPC + t * P + np.arange(P)
            S[ci, np.arange(P), cols] = dinv_pad[nodes] ** 2

    # idx SBUF layout: pos k -> [k % 16, k // 16], replicated 8x over partitions
    IDX = np.tile(IDXALL.reshape(NCORES, -1, 16).transpose(0, 2, 1), (1, 8, 1))

    x_pad = np.zeros((NPAD, C), dtype=ml_dtypes.bfloat16)
    x_pad[:N] = x.astype(ml_dtypes.bfloat16)

    meta = dict(
        nb=nb,
        blk_off=blk_off,
        gather_blk0=gather_blk0,
        gather_nblk=gather_nblk,
        totblk=totblk,
        s_off=s_off,
        tot_s=tot_s,
    )
    return x_pad, IDX, S.astype(ml_dtypes.bfloat16), WnT, b.reshape(1, C), meta


def _build(meta):
    """Build the SPMD Bass graph (shared across all 8 cores)."""
    nb = meta["nb"]
    blk_off = meta["blk_off"]
    gather_blk0 = meta["gather_blk0"]
    gather_nblk = meta["gather_nblk"]
    totblk = meta["totblk"]
    s_off = meta["s_off"]
    tot_s = meta["tot_s"]

    nc = bacc.Bacc("TRN2", target_bir_lowering=False, debug=False)

    x_d = nc.dram_tensor("x", [NPAD, C], mybir.dt.bfloat16, kind="ExternalInput")
    xself_d = nc.dram_tensor("xself", [NPC, C], mybir.dt.bfloat16, kind="ExternalInput")
    idx_d = nc.dram_tensor("idx", [P, totblk * 8], mybir.dt.int16, kind="ExternalInput")
    s_d = nc.dram_tensor("s", [P, tot_s * P], mybir.dt.bfloat16, kind="ExternalInput")
    wnT_d = nc.dram_tensor("wnT", [C, C], mybir.dt.float32, kind="ExternalInput")
    b_d = nc.dram_tensor("b", [1, C], mybir.dt.float32, kind="ExternalInput")
    out_d = nc.dram_tensor("out", [NPC, C], mybir.dt.float32, kind="ExternalOutput")

    nbc_max = int(gather_nblk.sum(axis=1).max())
    nbs_max = int((s_off[1:] - s_off[:-1]).max())

    with tile.TileContext(nc) as tc, ExitStack() as ctx:
        meta_p = ctx.enter_context(tc.tile_pool(name="meta", bufs=1))
        gat_p = ctx.enter_context(tc.tile_pool(name="gat", bufs=2))
        self_p = ctx.enter_context(tc.tile_pool(name="selfb", bufs=3))
        s_p = ctx.enter_context(tc.tile_pool(name="s", bufs=3))
        agg_p = ctx.enter_context(tc.tile_pool(name="agg", bufs=3))
        out_p = ctx.enter_context(tc.tile_pool(name="out", bufs=3))
        small_p = ctx.enter_context(tc.tile_pool(name="small", bufs=12))
        trash_p = ctx.enter_context(tc.tile_pool(name="trash", bufs=2))
        ps_agg = ctx.enter_context(tc.tile_pool(name="ps_agg", bufs=3, space="PSUM"))
        ps_out = ctx.enter_context(tc.tile_pool(name="ps_out", bufs=3, space="PSUM"))

        idx_sb = meta_p.tile([P, totblk * 8], mybir.dt.int16)
        nc.sync.dma_start(idx_sb[:], idx_d[:])
        wnT_sb = meta_p.tile([C, C], mybir.dt.float32)
        nc.sync.dma_start(wnT_sb[:], wnT_d[:])
        b_sb = meta_p.tile([1, C], mybir.dt.float32)
        nc.sync.dma_start(b_sb[:], b_d[:])
        ones_sb = meta_p.tile([1, C], mybir.dt.float32)
        nc.vector.memset(ones_sb[:], 1.0)
        eps_sb = meta_p.tile([P, 1], mybir.dt.float32)
        nc.vector.memset(eps_sb[:], EPS_IN)
        # batched per-tile row-stats (pass 2 consumes them)
        po_all = meta_p.tile([P, TPC * P], mybir.dt.float32)
        s1_all = meta_p.tile([P, TPC], mybir.dt.float32)
        ssq_all = meta_p.tile([P, TPC], mybir.dt.float32)

        x_lo = x_d[0:HALF, :]
        x_hi = x_d[HALF:NPAD, :]

        for ci in range(NCHUNKS):
            t0 = ci * CHUNK_TILES
            t1 = min(t0 + CHUNK_TILES, TPC)
            cblk0 = int(gather_blk0[ci, 0])
            gat_sb = gat_p.tile([P, nbc_max, P], mybir.dt.bfloat16, tag="gat")
            for h, src_ap in ((0, x_lo), (1, x_hi)):
                nblk_g = int(gather_nblk[ci, h])
                if nblk_g == 0:
                    continue
                nidx = nblk_g * P
                g0 = int(gather_blk0[ci, h]) - cblk0
                ic0 = int(gather_blk0[ci, h]) * 8
                nc.gpsimd.dma_gather(
                    out_ap=gat_sb[:, g0 : g0 + nblk_g, :],
                    in_ap=src_ap,
                    idxs_ap=idx_sb[:, ic0 : ic0 + nidx // 16],
                    num_idxs=nidx,
                    num_idxs_reg=nidx,
                    elem_size=C,
                    single_packet=False,
                )

            for t in range(t0, t1):
                nbs = int(s_off[t + 1] - s_off[t])
                ss = s_p.tile([P, nbs_max * P], mybir.dt.bfloat16, tag="ss")
                nc.sync.dma_start(
                    ss[:, : nbs * P],
                    s_d[:, int(s_off[t]) * P : int(s_off[t + 1]) * P],
                )
                # self rows: per-core slice of x (contiguous static load)
                g_self = self_p.tile([P, C], mybir.dt.bfloat16, tag="gs")
                nc.sync.dma_start(g_self[:], xself_d[t * P : (t + 1) * P, :])

                ngb = int(nb[t, 0] + nb[t, 1])
                pt = ps_agg.tile([P, P], mybir.dt.float32)
                nc.tensor.matmul(
                    pt[:], lhsT=g_self[:], rhs=ss[:, 0:P], start=True, stop=(ngb == 0)
                )
                for j in range(ngb):
                    gcol = (
                        int(blk_off[t, 0]) + j
                        if j < int(nb[t, 0])
                        else int(blk_off[t, 1]) + (j - int(nb[t, 0]))
                    )
                    nc.tensor.matmul(
                        pt[:],
                        lhsT=gat_sb[:, gcol - cblk0, :],
                        rhs=ss[:, (1 + j) * P : (2 + j) * P],
                        start=False,
                        stop=(j == ngb - 1),
                    )

                agg_sb = agg_p.tile([P, P], mybir.dt.float32)
                nc.scalar.copy(agg_sb[:], pt[:])

                po = ps_out.tile([P, P], mybir.dt.float32)
                nc.tensor.matmul(po[:], lhsT=agg_sb[:], rhs=wnT_sb[:], start=True, stop=False)
                nc.tensor.matmul(po[:], lhsT=ones_sb[:], rhs=b_sb[:], start=False, stop=True)

                # pass 1 epilogue: stash po + row sums / sumsq (batched math later)
                sqt = trash_p.tile([P, P], mybir.dt.float32, tag="sqt")
                nc.scalar.activation(
                    out=sqt[:], in_=po[:],
                    func=mybir.ActivationFunctionType.Square,
                    accum_out=ssq_all[:, t : t + 1],
                )
                nc.scalar.activation(
                    out=po_all[:, t * P : (t + 1) * P], in_=po[:],
                    func=mybir.ActivationFunctionType.Copy,
                    accum_out=s1_all[:, t : t + 1],
                )

        # --- batched InstanceNorm stats for all 49 tiles ---
        negmean = meta_p.tile([P, TPC], mybir.dt.float32)
        nc.vector.tensor_scalar(
            out=negmean[:], in0=s1_all[:], scalar1=-1.0 / C, scalar2=None,
            op0=mybir.AluOpType.mult,
        )
        msq = meta_p.tile([P, TPC], mybir.dt.float32)
        nc.vector.tensor_tensor(
            out=msq[:], in0=negmean[:], in1=negmean[:], op=mybir.AluOpType.mult
        )
        v1 = meta_p.tile([P, TPC], mybir.dt.float32)
        nc.vector.tensor_scalar(
            out=v1[:], in0=ssq_all[:], scalar1=1.0 / C, scalar2=None,
            op0=mybir.AluOpType.mult,
        )
        var = meta_p.tile([P, TPC], mybir.dt.float32)
        nc.vector.tensor_tensor(
            out=var[:], in0=v1[:], in1=msq[:], op=mybir.AluOpType.subtract
        )
        std = meta_p.tile([P, TPC], mybir.dt.float32)
        nc.scalar.activation(
            out=std[:], in_=var[:],
            func=mybir.ActivationFunctionType.Sqrt,
            bias=eps_sb[:], scale=1.0,
        )
        rstd = meta_p.tile([P, TPC], mybir.dt.float32)
        nc.vector.reciprocal(out=rstd[:], in_=std[:])
        negmr = meta_p.tile([P, TPC], mybir.dt.float32)
        nc.vector.tensor_tensor(
            out=negmr[:], in0=negmean[:], in1=rstd[:], op=mybir.AluOpType.mult
        )
        rstd02 = meta_p.tile([P, TPC], mybir.dt.float32)
        nc.scalar.activation(
            out=rstd02[:], in_=rstd[:],
            func=mybir.ActivationFunctionType.Copy, scale=0.2,
        )
        negmr02 = meta_p.tile([P, TPC], mybir.dt.float32)
        nc.scalar.activation(
            out=negmr02[:], in_=negmr[:],
            func=mybir.ActivationFunctionType.Copy, scale=0.2,
        )

        # --- pass 2: normalize + LeakyReLU + store ---
        for t in range(TPC):
            po_s = po_all[:, t * P : (t + 1) * P]
            normed = out_p.tile([P, P], mybir.dt.float32, tag="normed")
            nc.scalar.activation(
                out=normed[:], in_=po_s,
                func=mybir.ActivationFunctionType.Identity,
                bias=negmr[:, t : t + 1], scale=rstd[:, t : t + 1],
            )
            scaled = out_p.tile([P, P], mybir.dt.float32, tag="scaled")
            nc.scalar.activation(
                out=scaled[:], in_=po_s,
                func=mybir.ActivationFunctionType.Identity,
                bias=negmr02[:, t : t + 1], scale=rstd02[:, t : t + 1],
            )
            final = out_p.tile([P, P], mybir.dt.float32, tag="final")
            nc.vector.tensor_tensor(
                out=final[:], in0=normed[:], in1=scaled[:],
                op=mybir.AluOpType.max,
            )
            nc.sync.dma_start(out_d[t * P : (t + 1) * P, :], final[:])

    nc.compile()
    return nc


def _make_in_maps(x_pad, IDX, S, WnT, bvec):
    return [
        {
            "x": x_pad,
            "xself": np.ascontiguousarray(x_pad[i * NPC : (i + 1) * NPC]),
            "idx": np.ascontiguousarray(IDX[i]),
            "s": np.ascontiguousarray(S[i]),
            "wnT": WnT,
            "b": bvec,
        }
        for i in range(NCORES)
    ]


def kernel(x, edge_index, W, b, u):
    x_pad, IDX, S, WnT, bvec, meta = _preprocess(x, edge_index, W, b, u)
    nc = _build(meta)
    in_maps = _make_in_maps(x_pad, IDX, S, WnT, bvec)

    # The axon terminal can be transiently unavailable right after a prior
    # process's teardown; retry with backoff.
    import time

    last_err = None
    for attempt in range(6):
        try:
            res = run_bass_kernel_spmd(nc, in_maps, list(range(NCORES)))
            break
        except Exception as e:  # noqa: BLE001
            last_err = e
            time.sleep(45)
    else:
        raise last_err
    shards = [np.asarray(res.results[i]["out"]) for i in range(NCORES)]
    out = np.concatenate(shards, axis=0)[:N]
    return out.astype(np.float32)


# revision 14
# speedup vs baseline: 1.0451x; 1.0451x over previous
"""Trainium2 Bass kernel for GCNBlock (spectral-norm linear + GCN aggregation +
InstanceNorm + LeakyReLU) distributed across 8 NeuronCores.

v2 strategy (evolved from the dma_gather baseline after trace analysis):
  - out = (A @ x) @ WnT per dst tile; dst nodes sharded 8 ways (49 tiles of
    128 per core).
  - Gather of x rows per edge stays on gpsimd dma_gather (the only
    descriptor-rate-viable indexed path), edges partitioned by dst and
    sorted by src, split by int16 index halves, chunked 7 tiles per gather.
  - Self loops are NOT gathered: each tile's own x rows are a contiguous
    static HWDGE dma_start; their diag(coef) scatter block is part of S.
  - The one-hot scatter matrices S (including coef) are built ON HOST and
    streamed in via sync-engine DMA — the DVE tensor_scalar build used by the
    baseline costs ~2.4us per block (per-partition scalar-pointer reads) and
    was a co-bottleneck with the gather.
  - InstanceNorm epilogue avoids bn_stats/bn_aggr (1.8/4.5us per call) and
    per-partition-pointer DVE ops: ACT Square+accum gives sum(x^2), DVE
    tensor_reduce gives sum(x), tiny [128,1] ops produce rstd and
    -mean*rstd, and one ACT activation(Lrelu, scale=rstd, bias=-mean*rstd,
    alpha=0.2) applies normalize+LeakyReLU fused.
"""

import numpy as np
import ml_dtypes
from contextlib import ExitStack

import concourse.tile as tile
from concourse import bacc, mybir
from concourse.bass_utils import run_bass_kernel_spmd

# Problem constants (hardcoded per spec)
N, E, C = 50000, 800000, 128
P = 128
NCORES = 8
TPC = 49                # dst tiles per core
NPC = TPC * P           # 6272 dst nodes per core
NPAD = NCORES * NPC     # 50176 padded node count
HALF = 32768            # int16 index split point
CHUNK_TILES = 7
NCHUNKS = -(-TPC // CHUNK_TILES)  # 7
EPS_IN = 1e-5


def _preprocess(x, edge_index, W, b, u):
    """Host-side prep: spectral norm, edge partitioning, S-matrix build."""
    x = np.asarray(x, dtype=np.float32)
    ei = np.asarray(edge_index)
    W = np.asarray(W, dtype=np.float32)
    b = np.asarray(b, dtype=np.float32)
    u = np.asarray(u, dtype=np.float32)

    # --- spectral norm (one power iteration), matches reference ---
    eps = np.float32(1e-12)
    v = (W.T @ u).astype(np.float32)
    v = v / (np.float32(np.linalg.norm(v)) + eps)
    Wv = (W @ v).astype(np.float32)
    u2 = Wv / (np.float32(np.linalg.norm(Wv)) + eps)
    sigma = np.float32(u2 @ Wv)
    WnT = np.ascontiguousarray((W / sigma).T, dtype=np.float32)  # [cin, cout]

    src = ei[0].astype(np.int64)
    dst = ei[1].astype(np.int64)

    # --- degrees / coefficients (with self loops) ---
    deg = (np.bincount(dst, minlength=N) + 1).astype(np.float32)
    dinv = (1.0 / np.sqrt(deg)).astype(np.float32)
    dinv_pad = np.ones(NPAD, dtype=np.float32)
    dinv_pad[:N] = dinv
    coef = dinv[src] * dinv[dst]

    # --- group regular edges by (core, tile, src-half), sorted by src ---
    core = dst // NPC
    tile_g = (dst % NPC) // P
    dstloc = (dst % P).astype(np.int64)
    half = (src >= HALF).astype(np.int64)
    key = ((core * TPC + tile_g) * 2 + half).astype(np.int64)
    NG = NCORES * TPC * 2
    # sort by (key, src) so each group's gather addresses ascend
    order = np.lexsort((src, key))
    counts = np.bincount(key, minlength=NG)
    starts = np.zeros(NG + 1, dtype=np.int64)
    np.cumsum(counts, out=starts[1:])
    rank = np.arange(len(key), dtype=np.int64) - starts[key[order]]

    cnt3 = counts.reshape(NCORES, TPC, 2)
    nb = np.ceil(cnt3.max(axis=0) / P).astype(np.int64)  # [TPC, 2] gather blocks

    # block layout: chunk-major, then half, then tile within chunk
    blk_off = np.zeros((TPC, 2), dtype=np.int64)
    gather_blk0 = np.zeros((NCHUNKS, 2), dtype=np.int64)
    gather_nblk = np.zeros((NCHUNKS, 2), dtype=np.int64)
    pos = 0
    for c in range(NCHUNKS):
        t0 = c * CHUNK_TILES
        t1 = min(t0 + CHUNK_TILES, TPC)
        for h in range(2):
            gather_blk0[c, h] = pos
            for t in range(t0, t1):
                blk_off[t, h] = pos
                pos += nb[t, h]
            gather_nblk[c, h] = pos - gather_blk0[c, h]
    totblk = pos

    # gather indices (int16, wrapped) + S matrices for gather blocks
    IDXALL = np.zeros((NCORES, totblk * P), dtype=np.int16)
    # S layout: per tile t: [1 self block | nb[t,0] low blocks | nb[t,1] high]
    nbt = nb.sum(axis=1) + 1          # total S blocks per tile
    s_off = np.zeros(TPC + 1, dtype=np.int64)
    np.cumsum(nbt, out=s_off[1:])
    tot_s = int(s_off[-1])
    S = np.zeros((NCORES, P, tot_s * P), dtype=np.float32)

    o_core = core[order]
    o_tile = tile_g[order]
    o_half = half[order]
    o_blk = blk_off[o_tile, o_half] + rank // P
    o_slot = rank % P
    # S column index for edge: tile base + (1 + local block) * P + dstloc
    loc_blk = o_blk - blk_off[o_tile, 0]          # local gather-block id within tile
    # for high half, local id continues after low blocks:
    loc_blk = np.where(o_half == 1, nb[o_tile, 0] + (o_blk - blk_off[o_tile, 1]), loc_blk)
    s_col = (s_off[o_tile] + 1 + loc_blk) * P + dstloc[order]

    IDXALL[o_core, o_blk * P + o_slot] = (src[order] - o_half * HALF).astype(np.int16)
    S[o_core, o_slot, s_col] = coef[order]

    # self-loop diag blocks
    for t in range(TPC):
        cols = (s_off[t] * P) + np.arange(P)
        for ci in range(NCORES):
            nodes = ci * NPC + t * P + np.arange(P)
            S[ci, np.arange(P), cols] = dinv_pad[nodes] ** 2

    # idx SBUF layout: pos k -> [k % 16, k // 16], replicated 8x over partitions
    IDX = np.tile(IDXALL.reshape(NCORES, -1, 16).transpose(0, 2, 1), (1, 8, 1))

    x_pad = np.zeros((NPAD, C), dtype=ml_dtypes.bfloat16)
    x_pad[:N] = x.astype(ml_dtypes.bfloat16)

    meta = dict(
        nb=nb,
        blk_off=blk_off,
        gather_blk0=gather_blk0,
        gather_nblk=gather_nblk,
        totblk=totblk,
        s_off=s_off,
        tot_s=tot_s,
    )
    return x_pad, IDX, S.astype(ml_dtypes.bfloat16), WnT, b.reshape(1, C), meta


def _build(meta):
    """Build the SPMD Bass graph (shared across all 8 cores)."""
    nb = meta["nb"]
    blk_off = meta["blk_off"]
    gather_blk0 = meta["gather_blk0"]
    gather_nblk = meta["gather_nblk"]
    totblk = meta["totblk"]
    s_off = meta["s_off"]
    tot_s = meta["tot_s"]

    nc = bacc.Bacc("TRN2", target_bir_lowering=False, debug=False)

    x_d = nc.dram_tensor("x", [NPAD, C], mybir.dt.bfloat16, kind="ExternalInput")
    xself_d = nc.dram_tensor("xself", [NPC, C], mybir.dt.bfloat16, kind="ExternalInput")
    idx_d = nc.dram_tensor("idx", [P, totblk * 8], mybir.dt.int16, kind="ExternalInput")
    s_d = nc.dram_tensor("s", [P, tot_s * P], mybir.dt.bfloat16, kind="ExternalInput")
    wnT_d = nc.dram_tensor("wnT", [C, C], mybir.dt.float32, kind="ExternalInput")
    b_d = nc.dram_tensor("b", [1, C], mybir.dt.float32, kind="ExternalInput")
    out_d = nc.dram_tensor("out", [NPC, C], mybir.dt.float32, kind="ExternalOutput")

    nbc_max = int(gather_nblk.sum(axis=1).max())
    nbs_max = int((s_off[1:] - s_off[:-1]).max())

    with tile.TileContext(nc) as tc, ExitStack() as ctx:
        meta_p = ctx.enter_context(tc.tile_pool(name="meta", bufs=1))
        gat_p = ctx.enter_context(tc.tile_pool(name="gat", bufs=3))
        self_p = ctx.enter_context(tc.tile_pool(name="selfb", bufs=3))
        s_p = ctx.enter_context(tc.tile_pool(name="s", bufs=3))
        agg_p = ctx.enter_context(tc.tile_pool(name="agg", bufs=3))
        out_p = ctx.enter_context(tc.tile_pool(name="out", bufs=3))
        small_p = ctx.enter_context(tc.tile_pool(name="small", bufs=12))
        trash_p = ctx.enter_context(tc.tile_pool(name="trash", bufs=2))
        ps_agg = ctx.enter_context(tc.tile_pool(name="ps_agg", bufs=3, space="PSUM"))
        ps_out = ctx.enter_context(tc.tile_pool(name="ps_out", bufs=3, space="PSUM"))

        idx_sb = meta_p.tile([P, totblk * 8], mybir.dt.int16)
        nc.sync.dma_start(idx_sb[:], idx_d[:])
        wnT_sb = meta_p.tile([C, C], mybir.dt.float32)
        nc.sync.dma_start(wnT_sb[:], wnT_d[:])
        b_sb = meta_p.tile([1, C], mybir.dt.float32)
        nc.sync.dma_start(b_sb[:], b_d[:])
        ones_sb = meta_p.tile([1, C], mybir.dt.float32)
        nc.vector.memset(ones_sb[:], 1.0)
        eps_sb = meta_p.tile([P, 1], mybir.dt.float32)
        nc.vector.memset(eps_sb[:], EPS_IN)

        x_lo = x_d[0:HALF, :]
        x_hi = x_d[HALF:NPAD, :]

        for ci in range(NCHUNKS):
            t0 = ci * CHUNK_TILES
            t1 = min(t0 + CHUNK_TILES, TPC)
            nt = t1 - t0
            # per-chunk stats + po stash (pass 2 runs per chunk, overlapped)
            po_c = agg_p.tile([P, CHUNK_TILES * P], mybir.dt.float32, tag="poc")
            s1_c = small_p.tile([P, CHUNK_TILES], mybir.dt.float32, tag="s1c")
            ssq_c = small_p.tile([P, CHUNK_TILES], mybir.dt.float32, tag="ssqc")
            cblk0 = int(gather_blk0[ci, 0])
            gat_sb = gat_p.tile([P, nbc_max, P], mybir.dt.bfloat16, tag="gat")
            for h, src_ap in ((0, x_lo), (1, x_hi)):
                nblk_g = int(gather_nblk[ci, h])
                if nblk_g == 0:
                    continue
                nidx = nblk_g * P
                g0 = int(gather_blk0[ci, h]) - cblk0
                ic0 = int(gather_blk0[ci, h]) * 8
                nc.gpsimd.dma_gather(
                    out_ap=gat_sb[:, g0 : g0 + nblk_g, :],
                    in_ap=src_ap,
                    idxs_ap=idx_sb[:, ic0 : ic0 + nidx // 16],
                    num_idxs=nidx,
                    num_idxs_reg=nidx,
                    elem_size=C,
                    single_packet=False,
                )

            for t in range(t0, t1):
                nbs = int(s_off[t + 1] - s_off[t])
                ss = s_p.tile([P, nbs_max * P], mybir.dt.bfloat16, tag="ss")
                nc.sync.dma_start(
                    ss[:, : nbs * P],
                    s_d[:, int(s_off[t]) * P : int(s_off[t + 1]) * P],
                )
                # self rows: per-core slice of x (contiguous static load)
                g_self = self_p.tile([P, C], mybir.dt.bfloat16, tag="gs")
                nc.sync.dma_start(g_self[:], xself_d[t * P : (t + 1) * P, :])

                ngb = int(nb[t, 0] + nb[t, 1])
                pt = ps_agg.tile([P, P], mybir.dt.float32)
                nc.tensor.matmul(
                    pt[:], lhsT=g_self[:], rhs=ss[:, 0:P], start=True, stop=(ngb == 0)
                )
                for j in range(ngb):
                    gcol = (
                        int(blk_off[t, 0]) + j
                        if j < int(nb[t, 0])
                        else int(blk_off[t, 1]) + (j - int(nb[t, 0]))
                    )
                    nc.tensor.matmul(
                        pt[:],
                        lhsT=gat_sb[:, gcol - cblk0, :],
                        rhs=ss[:, (1 + j) * P : (2 + j) * P],
                        start=False,
                        stop=(j == ngb - 1),
                    )

                agg_sb = agg_p.tile([P, P], mybir.dt.float32)
                nc.scalar.copy(agg_sb[:], pt[:])

                po = ps_out.tile([P, P], mybir.dt.float32)
                nc.tensor.matmul(po[:], lhsT=agg_sb[:], rhs=wnT_sb[:], start=True, stop=False)
                nc.tensor.matmul(po[:], lhsT=ones_sb[:], rhs=b_sb[:], start=False, stop=True)

                # pass 1 epilogue: stash po + row sums / sumsq (batched math later)
                tl = t - t0
                sqt = trash_p.tile([P, P], mybir.dt.float32, tag="sqt")
                nc.scalar.activation(
                    out=sqt[:], in_=po[:],
                    func=mybir.ActivationFunctionType.Square,
                    accum_out=ssq_c[:, tl : tl + 1],
                )
                nc.scalar.activation(
                    out=po_c[:, tl * P : (tl + 1) * P], in_=po[:],
                    func=mybir.ActivationFunctionType.Copy,
                    accum_out=s1_c[:, tl : tl + 1],
                )

            # --- batched InstanceNorm stats for this chunk's tiles ---
            negmean = small_p.tile([P, CHUNK_TILES], mybir.dt.float32, tag="nm")
            nc.vector.tensor_scalar(
                out=negmean[:, :nt], in0=s1_c[:, :nt], scalar1=-1.0 / C,
                scalar2=None, op0=mybir.AluOpType.mult,
            )
            msq = small_p.tile([P, CHUNK_TILES], mybir.dt.float32, tag="msq")
            nc.vector.tensor_tensor(
                out=msq[:, :nt], in0=negmean[:, :nt], in1=negmean[:, :nt],
                op=mybir.AluOpType.mult,
            )
            v1 = small_p.tile([P, CHUNK_TILES], mybir.dt.float32, tag="v1")
            nc.vector.tensor_scalar(
                out=v1[:, :nt], in0=ssq_c[:, :nt], scalar1=1.0 / C,
                scalar2=None, op0=mybir.AluOpType.mult,
            )
            var = small_p.tile([P, CHUNK_TILES], mybir.dt.float32, tag="var")
            nc.vector.tensor_tensor(
                out=var[:, :nt], in0=v1[:, :nt], in1=msq[:, :nt],
                op=mybir.AluOpType.subtract,
            )
            std = small_p.tile([P, CHUNK_TILES], mybir.dt.float32, tag="std")
            nc.scalar.activation(
                out=std[:, :nt], in_=var[:, :nt],
                func=mybir.ActivationFunctionType.Sqrt,
                bias=eps_sb[:], scale=1.0,
            )
            rstd = small_p.tile([P, CHUNK_TILES], mybir.dt.float32, tag="rstd")
            nc.vector.reciprocal(out=rstd[:, :nt], in_=std[:, :nt])
            negmr = small_p.tile([P, CHUNK_TILES], mybir.dt.float32, tag="negmr")
            nc.vector.tensor_tensor(
                out=negmr[:, :nt], in0=negmean[:, :nt], in1=rstd[:, :nt],
                op=mybir.AluOpType.mult,
            )
            rstd02 = small_p.tile([P, CHUNK_TILES], mybir.dt.float32, tag="rstd02")
            nc.scalar.activation(
                out=rstd02[:, :nt], in_=rstd[:, :nt],
                func=mybir.ActivationFunctionType.Copy, scale=0.2,
            )
            negmr02 = small_p.tile([P, CHUNK_TILES], mybir.dt.float32, tag="negmr02")
            nc.scalar.activation(
                out=negmr02[:, :nt], in_=negmr[:, :nt],
                func=mybir.ActivationFunctionType.Copy, scale=0.2,
            )

            # --- pass 2: normalize + LeakyReLU + store (this chunk) ---
            for t in range(t0, t1):
                tl = t - t0
                po_s = po_c[:, tl * P : (tl + 1) * P]
                normed = out_p.tile([P, P], mybir.dt.float32, tag="normed")
                nc.scalar.activation(
                    out=normed[:], in_=po_s,
                    func=mybir.ActivationFunctionType.Identity,
                    bias=negmr[:, tl : tl + 1], scale=rstd[:, tl : tl + 1],
                )
                scaled = out_p.tile([P, P], mybir.dt.float32, tag="scaled")
                nc.scalar.activation(
                    out=scaled[:], in_=po_s,
                    func=mybir.ActivationFunctionType.Identity,
                    bias=negmr02[:, tl : tl + 1], scale=rstd02[:, tl : tl + 1],
                )
                final = out_p.tile([P, P], mybir.dt.float32, tag="final")
                nc.vector.tensor_tensor(
                    out=final[:], in0=normed[:], in1=scaled[:],
                    op=mybir.AluOpType.max,
                )
                nc.sync.dma_start(out_d[t * P : (t + 1) * P, :], final[:])

    nc.compile()
    return nc


def _make_in_maps(x_pad, IDX, S, WnT, bvec):
    return [
        {
            "x": x_pad,
            "xself": np.ascontiguousarray(x_pad[i * NPC : (i + 1) * NPC]),
            "idx": np.ascontiguousarray(IDX[i]),
            "s": np.ascontiguousarray(S[i]),
            "wnT": WnT,
            "b": bvec,
        }
        for i in range(NCORES)
    ]


def kernel(x, edge_index, W, b, u):
    x_pad, IDX, S, WnT, bvec, meta = _preprocess(x, edge_index, W, b, u)
    nc = _build(meta)
    in_maps = _make_in_maps(x_pad, IDX, S, WnT, bvec)

    # The axon terminal can be transiently unavailable right after a prior
    # process's teardown; retry with backoff.
    import time

    last_err = None
    for attempt in range(6):
        try:
            res = run_bass_kernel_spmd(nc, in_maps, list(range(NCORES)))
            break
        except Exception as e:  # noqa: BLE001
            last_err = e
            time.sleep(45)
    else:
        raise last_err
    shards = [np.asarray(res.results[i]["out"]) for i in range(NCORES)]
    out = np.concatenate(shards, axis=0)[:N]
    return out.astype(np.float32)


# revision 15
# speedup vs baseline: 1.1601x; 1.1100x over previous
"""Trainium2 Bass kernel for GCNBlock (spectral-norm linear + GCN aggregation +
InstanceNorm + LeakyReLU) distributed across 8 NeuronCores.

v2 strategy (evolved from the dma_gather baseline after trace analysis):
  - out = (A @ x) @ WnT per dst tile; dst nodes sharded 8 ways (49 tiles of
    128 per core).
  - Gather of x rows per edge stays on gpsimd dma_gather (the only
    descriptor-rate-viable indexed path), edges partitioned by dst and
    sorted by src, split by int16 index halves, chunked 7 tiles per gather.
  - Self loops are NOT gathered: each tile's own x rows are a contiguous
    static HWDGE dma_start; their diag(coef) scatter block is part of S.
  - The one-hot scatter matrices S (including coef) are built ON HOST and
    streamed in via sync-engine DMA — the DVE tensor_scalar build used by the
    baseline costs ~2.4us per block (per-partition scalar-pointer reads) and
    was a co-bottleneck with the gather.
  - InstanceNorm epilogue avoids bn_stats/bn_aggr (1.8/4.5us per call) and
    per-partition-pointer DVE ops: ACT Square+accum gives sum(x^2), DVE
    tensor_reduce gives sum(x), tiny [128,1] ops produce rstd and
    -mean*rstd, and one ACT activation(Lrelu, scale=rstd, bias=-mean*rstd,
    alpha=0.2) applies normalize+LeakyReLU fused.
"""

import numpy as np
import ml_dtypes
from contextlib import ExitStack

import concourse.tile as tile
from concourse import bacc, mybir
from concourse.bass_utils import run_bass_kernel_spmd

# Problem constants (hardcoded per spec)
N, E, C = 50000, 800000, 128
P = 128
NCORES = 8
TPC = 49                # dst tiles per core
NPC = TPC * P           # 6272 dst nodes per core
NPAD = NCORES * NPC     # 50176 padded node count
HALF = 32768            # int16 index split point
CHUNK_TILES = 7
NCHUNKS = -(-TPC // CHUNK_TILES)  # 7
EPS_IN = 1e-5


def _preprocess(x, edge_index, W, b, u):
    """Host-side prep: spectral norm, edge partitioning, S-matrix build."""
    x = np.asarray(x, dtype=np.float32)
    ei = np.asarray(edge_index)
    W = np.asarray(W, dtype=np.float32)
    b = np.asarray(b, dtype=np.float32)
    u = np.asarray(u, dtype=np.float32)

    # --- spectral norm (one power iteration), matches reference ---
    eps = np.float32(1e-12)
    v = (W.T @ u).astype(np.float32)
    v = v / (np.float32(np.linalg.norm(v)) + eps)
    Wv = (W @ v).astype(np.float32)
    u2 = Wv / (np.float32(np.linalg.norm(Wv)) + eps)
    sigma = np.float32(u2 @ Wv)
    WnT = np.ascontiguousarray((W / sigma).T, dtype=np.float32)  # [cin, cout]

    src = ei[0].astype(np.int64)
    dst = ei[1].astype(np.int64)

    # --- degrees / coefficients (with self loops) ---
    deg = (np.bincount(dst, minlength=N) + 1).astype(np.float32)
    dinv = (1.0 / np.sqrt(deg)).astype(np.float32)
    dinv_pad = np.ones(NPAD, dtype=np.float32)
    dinv_pad[:N] = dinv
    coef = dinv[src] * dinv[dst]

    # --- group regular edges by (core, tile, src-half), sorted by src ---
    core = dst // NPC
    tile_g = (dst % NPC) // P
    dstloc = (dst % P).astype(np.int64)
    half = (src >= HALF).astype(np.int64)
    key = ((core * TPC + tile_g) * 2 + half).astype(np.int64)
    NG = NCORES * TPC * 2
    # sort by (key, src) so each group's gather addresses ascend
    order = np.lexsort((src, key))
    counts = np.bincount(key, minlength=NG)
    starts = np.zeros(NG + 1, dtype=np.int64)
    np.cumsum(counts, out=starts[1:])
    rank = np.arange(len(key), dtype=np.int64) - starts[key[order]]

    cnt3 = counts.reshape(NCORES, TPC, 2)
    nb = np.ceil(cnt3.max(axis=0) / P).astype(np.int64)  # [TPC, 2] gather blocks

    # block layout: chunk-major, then half, then tile within chunk
    blk_off = np.zeros((TPC, 2), dtype=np.int64)
    gather_blk0 = np.zeros((NCHUNKS, 2), dtype=np.int64)
    gather_nblk = np.zeros((NCHUNKS, 2), dtype=np.int64)
    pos = 0
    for c in range(NCHUNKS):
        t0 = c * CHUNK_TILES
        t1 = min(t0 + CHUNK_TILES, TPC)
        for h in range(2):
            gather_blk0[c, h] = pos
            for t in range(t0, t1):
                blk_off[t, h] = pos
                pos += nb[t, h]
            gather_nblk[c, h] = pos - gather_blk0[c, h]
    totblk = pos

    # gather indices (int16, wrapped) + S matrices for gather blocks
    IDXALL = np.zeros((NCORES, totblk * P), dtype=np.int16)
    # S layout: per tile t: [1 self block | nb[t,0] low blocks | nb[t,1] high]
    nbt = nb.sum(axis=1) + 1          # total S blocks per tile
    s_off = np.zeros(TPC + 1, dtype=np.int64)
    np.cumsum(nbt, out=s_off[1:])
    tot_s = int(s_off[-1])
    S = np.zeros((NCORES, P, tot_s * P), dtype=np.float32)

    o_core = core[order]
    o_tile = tile_g[order]
    o_half = half[order]
    o_blk = blk_off[o_tile, o_half] + rank // P
    o_slot = rank % P
    # S column index for edge: tile base + (1 + local block) * P + dstloc
    loc_blk = o_blk - blk_off[o_tile, 0]          # local gather-block id within tile
    # for high half, local id continues after low blocks:
    loc_blk = np.where(o_half == 1, nb[o_tile, 0] + (o_blk - blk_off[o_tile, 1]), loc_blk)
    s_col = (s_off[o_tile] + 1 + loc_blk) * P + dstloc[order]

    IDXALL[o_core, o_blk * P + o_slot] = (src[order] - o_half * HALF).astype(np.int16)
    S[o_core, o_slot, s_col] = coef[order]

    # self-loop diag blocks
    for t in range(TPC):
        cols = (s_off[t] * P) + np.arange(P)
        for ci in range(NCORES):
            nodes = ci * NPC + t * P + np.arange(P)
            S[ci, np.arange(P), cols] = dinv_pad[nodes] ** 2

    # idx SBUF layout: pos k -> [k % 16, k // 16], replicated 8x over partitions
    IDX = np.tile(IDXALL.reshape(NCORES, -1, 16).transpose(0, 2, 1), (1, 8, 1))

    x_pad = np.zeros((NPAD, C), dtype=ml_dtypes.bfloat16)
    x_pad[:N] = x.astype(ml_dtypes.bfloat16)

    meta = dict(
        nb=nb,
        blk_off=blk_off,
        gather_blk0=gather_blk0,
        gather_nblk=gather_nblk,
        totblk=totblk,
        s_off=s_off,
        tot_s=tot_s,
    )
    return x_pad, IDX, S.astype(ml_dtypes.bfloat16), WnT, b.reshape(1, C), meta


def _build(meta):
    """Build the SPMD Bass graph (shared across all 8 cores)."""
    nb = meta["nb"]
    blk_off = meta["blk_off"]
    gather_blk0 = meta["gather_blk0"]
    gather_nblk = meta["gather_nblk"]
    totblk = meta["totblk"]
    s_off = meta["s_off"]
    tot_s = meta["tot_s"]

    nc = bacc.Bacc("TRN2", target_bir_lowering=False, debug=False)

    x_d = nc.dram_tensor("x", [NPAD, C], mybir.dt.bfloat16, kind="ExternalInput")
    xself_d = nc.dram_tensor("xself", [NPC, C], mybir.dt.bfloat16, kind="ExternalInput")
    idx_d = nc.dram_tensor("idx", [P, totblk * 8], mybir.dt.int16, kind="ExternalInput")
    s_d = nc.dram_tensor("s", [P, tot_s * P], mybir.dt.bfloat16, kind="ExternalInput")
    wnT_d = nc.dram_tensor("wnT", [C, C], mybir.dt.float32, kind="ExternalInput")
    b_d = nc.dram_tensor("b", [1, C], mybir.dt.float32, kind="ExternalInput")
    out_d = nc.dram_tensor("out", [NPC, C], mybir.dt.float32, kind="ExternalOutput")

    nbc_max = int(gather_nblk.sum(axis=1).max())
    nbs_max = int((s_off[1:] - s_off[:-1]).max())

    with tile.TileContext(nc) as tc, ExitStack() as ctx:
        meta_p = ctx.enter_context(tc.tile_pool(name="meta", bufs=1))
        gat_p = ctx.enter_context(tc.tile_pool(name="gat", bufs=3))
        self_p = ctx.enter_context(tc.tile_pool(name="selfb", bufs=3))
        s_p = ctx.enter_context(tc.tile_pool(name="s", bufs=3))
        agg_p = ctx.enter_context(tc.tile_pool(name="agg", bufs=3))
        out_p = ctx.enter_context(tc.tile_pool(name="out", bufs=3))
        small_p = ctx.enter_context(tc.tile_pool(name="small", bufs=12))
        trash_p = ctx.enter_context(tc.tile_pool(name="trash", bufs=2))
        ps_agg = ctx.enter_context(tc.tile_pool(name="ps_agg", bufs=3, space="PSUM"))
        ps_out = ctx.enter_context(tc.tile_pool(name="ps_out", bufs=3, space="PSUM"))

        idx_sb = meta_p.tile([P, totblk * 8], mybir.dt.int16)
        nc.sync.dma_start(idx_sb[:], idx_d[:])
        wnT_sb = meta_p.tile([C, C], mybir.dt.float32)
        nc.sync.dma_start(wnT_sb[:], wnT_d[:])
        b_sb = meta_p.tile([1, C], mybir.dt.float32)
        nc.sync.dma_start(b_sb[:], b_d[:])
        ones_sb = meta_p.tile([1, C], mybir.dt.float32)
        nc.vector.memset(ones_sb[:], 1.0)
        eps_sb = meta_p.tile([P, 1], mybir.dt.float32)
        nc.vector.memset(eps_sb[:], EPS_IN)

        x_lo = x_d[0:HALF, :]
        x_hi = x_d[HALF:NPAD, :]

        for ci in range(NCHUNKS):
            t0 = ci * CHUNK_TILES
            t1 = min(t0 + CHUNK_TILES, TPC)
            cblk0 = int(gather_blk0[ci, 0])
            gat_sb = gat_p.tile([P, nbc_max, P], mybir.dt.bfloat16, tag="gat")
            for h, src_ap in ((0, x_lo), (1, x_hi)):
                nblk_g = int(gather_nblk[ci, h])
                if nblk_g == 0:
                    continue
                nidx = nblk_g * P
                g0 = int(gather_blk0[ci, h]) - cblk0
                ic0 = int(gather_blk0[ci, h]) * 8
                nc.gpsimd.dma_gather(
                    out_ap=gat_sb[:, g0 : g0 + nblk_g, :],
                    in_ap=src_ap,
                    idxs_ap=idx_sb[:, ic0 : ic0 + nidx // 16],
                    num_idxs=nidx,
                    num_idxs_reg=nidx,
                    elem_size=C,
                    single_packet=False,
                )

            for t in range(t0, t1):
                nbs = int(s_off[t + 1] - s_off[t])
                ss = s_p.tile([P, nbs_max * P], mybir.dt.bfloat16, tag="ss")
                nc.sync.dma_start(
                    ss[:, : nbs * P],
                    s_d[:, int(s_off[t]) * P : int(s_off[t + 1]) * P],
                )
                # self rows: per-core slice of x (contiguous static load)
                g_self = self_p.tile([P, C], mybir.dt.bfloat16, tag="gs")
                nc.sync.dma_start(g_self[:], xself_d[t * P : (t + 1) * P, :])

                ngb = int(nb[t, 0] + nb[t, 1])
                pt = ps_agg.tile([P, P], mybir.dt.float32)
                nc.tensor.matmul(
                    pt[:], lhsT=g_self[:], rhs=ss[:, 0:P], start=True, stop=(ngb == 0)
                )
                for j in range(ngb):
                    gcol = (
                        int(blk_off[t, 0]) + j
                        if j < int(nb[t, 0])
                        else int(blk_off[t, 1]) + (j - int(nb[t, 0]))
                    )
                    nc.tensor.matmul(
                        pt[:],
                        lhsT=gat_sb[:, gcol - cblk0, :],
                        rhs=ss[:, (1 + j) * P : (2 + j) * P],
                        start=False,
                        stop=(j == ngb - 1),
                    )

                agg_sb = agg_p.tile([P, P], mybir.dt.float32)
                nc.vector.tensor_copy(agg_sb[:], pt[:])

                po = ps_out.tile([P, P], mybir.dt.float32)
                nc.tensor.matmul(po[:], lhsT=agg_sb[:], rhs=wnT_sb[:], start=True, stop=False)
                nc.tensor.matmul(po[:], lhsT=ones_sb[:], rhs=b_sb[:], start=False, stop=True)

                # --- InstanceNorm + LeakyReLU epilogue (ACT-centric) ---
                ssq = small_p.tile([P, 1], mybir.dt.float32, tag="ssq")
                sqt = trash_p.tile([P, P], mybir.dt.float32, tag="sqt")
                nc.scalar.activation(
                    out=sqt[:], in_=po[:],
                    func=mybir.ActivationFunctionType.Square,
                    accum_out=ssq[:],
                )
                s1 = small_p.tile([P, 1], mybir.dt.float32, tag="s1")
                nc.vector.tensor_reduce(
                    out=s1[:], in_=po[:], axis=mybir.AxisListType.X,
                    op=mybir.AluOpType.add,
                )
                negmean = small_p.tile([P, 1], mybir.dt.float32, tag="nm")
                nc.vector.tensor_scalar(
                    out=negmean[:], in0=s1[:], scalar1=-1.0 / C, scalar2=None,
                    op0=mybir.AluOpType.mult,
                )
                msq = small_p.tile([P, 1], mybir.dt.float32, tag="msq")
                nc.vector.tensor_tensor(
                    out=msq[:], in0=negmean[:], in1=negmean[:], op=mybir.AluOpType.mult
                )
                v1 = small_p.tile([P, 1], mybir.dt.float32, tag="v1")
                nc.vector.tensor_scalar(
                    out=v1[:], in0=ssq[:], scalar1=1.0 / C, scalar2=None,
                    op0=mybir.AluOpType.mult,
                )
                var = small_p.tile([P, 1], mybir.dt.float32, tag="var")
                nc.vector.tensor_tensor(
                    out=var[:], in0=v1[:], in1=msq[:], op=mybir.AluOpType.subtract
                )
                std = small_p.tile([P, 1], mybir.dt.float32, tag="std")
                nc.scalar.activation(
                    out=std[:], in_=var[:],
                    func=mybir.ActivationFunctionType.Sqrt,
                    bias=eps_sb[:], scale=1.0,
                )
                rstd = small_p.tile([P, 1], mybir.dt.float32, tag="rstd")
                nc.vector.reciprocal(out=rstd[:], in_=std[:])
                negmr = small_p.tile([P, 1], mybir.dt.float32, tag="negmr")
                nc.vector.tensor_tensor(
                    out=negmr[:], in0=negmean[:], in1=rstd[:], op=mybir.AluOpType.mult
                )
                normed = out_p.tile([P, P], mybir.dt.float32, tag="normed")
                nc.scalar.activation(
                    out=normed[:], in_=po[:],
                    func=mybir.ActivationFunctionType.Identity,
                    bias=negmr[:], scale=rstd[:],
                )
                scaled = out_p.tile([P, P], mybir.dt.float32, tag="scaled")
                nc.vector.tensor_scalar(
                    out=scaled[:], in0=normed[:], scalar1=0.2, scalar2=None,
                    op0=mybir.AluOpType.mult,
                )
                final = out_p.tile([P, P], mybir.dt.float32, tag="final")
                nc.vector.tensor_tensor(
                    out=final[:], in0=normed[:], in1=scaled[:],
                    op=mybir.AluOpType.max,
                )
                nc.sync.dma_start(out_d[t * P : (t + 1) * P, :], final[:])

    nc.compile()
    return nc


def _make_in_maps(x_pad, IDX, S, WnT, bvec):
    return [
        {
            "x": x_pad,
            "xself": np.ascontiguousarray(x_pad[i * NPC : (i + 1) * NPC]),
            "idx": np.ascontiguousarray(IDX[i]),
            "s": np.ascontiguousarray(S[i]),
            "wnT": WnT,
            "b": bvec,
        }
        for i in range(NCORES)
    ]


def kernel(x, edge_index, W, b, u):
    x_pad, IDX, S, WnT, bvec, meta = _preprocess(x, edge_index, W, b, u)
    nc = _build(meta)
    in_maps = _make_in_maps(x_pad, IDX, S, WnT, bvec)

    # The axon terminal can be transiently unavailable right after a prior
    # process's teardown; retry with backoff.
    import time

    last_err = None
    for attempt in range(6):
        try:
            res = run_bass_kernel_spmd(nc, in_maps, list(range(NCORES)))
            break
        except Exception as e:  # noqa: BLE001
            last_err = e
            time.sleep(45)
    else:
        raise last_err
    shards = [np.asarray(res.results[i]["out"]) for i in range(NCORES)]
    out = np.concatenate(shards, axis=0)[:N]
    return out.astype(np.float32)
